# revision 18
# baseline (speedup 1.0000x reference)
"""Two-layer GraphConv (gather + segment-mean + linear + ReLU) x2 + sigmoid head,
distributed over 8 NeuronCores.

Sharding: destination nodes are partitioned across the 8 cores (12.5k each).
Host-side prep (pure index work): each core's edges are bucketed by
(src-chunk-of-25k, dst), each (chunk x dst-tile-of-128) run is padded to a
multiple of 128 with sentinel edges so all 8 cores share one SPMD program.

On device, per layer (bf16 data path, fp32 accumulation in PSUM):
  - node features live in DRAM as [N, 128] bf16 (64 real features + 64 pad)
    so one 256B dma_gather element fetches one row with no conversion step
  - dma_gather fetches rows via int16 chunk-local indices (4 swdge queues,
    1024-desc batches, 10 in flight; the whole idx/drel stream is preloaded
    into SBUF once and shared by both layers)
  - one-hot matrices are built on the vector engine in bf16
  - TensorE bf16 matmuls (lhsT=one-hot, rhs=gathered msgs[:, :64]) segment-sum
    into PSUM, accumulated per dst-tile into an fp32 SBUF accumulator
  - per-tile tails (1/deg scale on ACT, PE-transpose, fused W+bias bf16
    matmuls, ReLU) are emitted inline as soon as a tile's last run finishes,
    so they overlap the remaining gather/matmul stream; x1 writeback is
    chunked the same way so the AllGather starts early
  - AllGather of x1 (bf16, padded layout); layer 2 gathers from per-pass
    local-DRAM copies (reads from the Shared address space are ~70% slower)
  - layer-2 tail: ReLU with accumulated row-sum, sigmoid(scale*s+bias)

Measured on 8 trn2 cores: 1.97-2.05 ms HW exec (baseline fp32 dma-gather
implementation: 12.2 ms); rel err vs fp32 reference ~1.7e-5. The pacer is
the swdge gather itself (~2.8 us per 1024-desc batch = HBM random-read
bound); PE pipelines groups at ~44 ns within a sub-batch and is ~20% busy.
BSZ=2048 overflows the per-queue descriptor ring (~2.5k-4k desc cap) and
crashes the device - do not raise BSZ or msgs bufs beyond 10.
"""

import os
import sys

for _p in ("/opt/trn_rl_repo", "/opt/pypackages"):
    if _p not in sys.path and os.path.isdir(_p):
        sys.path.insert(0, _p)

import numpy as np
import ml_dtypes

from concourse import bacc, bass, mybir, tile
from concourse.bass_utils import run_bass_kernel_spmd

F32 = mybir.dt.float32
BF16 = mybir.dt.bfloat16
I16 = mybir.dt.int16
BF = ml_dtypes.bfloat16

TILE = 128
FPAD = 128  # padded feature row width (bf16) -> 256B gather element


def _cdiv(a, b):
    return (a + b - 1) // b


class Cfg:
    def __init__(self, N=100000, D=64, C=8, CH=25000, BSZ=1024, no_cc=False):
        self.no_cc = no_cc
        assert N % C == 0 and N % CH == 0
        assert CH <= 32768  # int16 gather indices
        assert BSZ % 128 == 0
        self.N, self.D, self.C, self.CH, self.BSZ = N, D, C, CH, BSZ
        self.NDST = N // C
        self.NT = _cdiv(self.NDST, TILE)
        self.NP = N // CH
        self.D2 = 32  # layer-2 output width


def plan_edges(edge_src, edge_dst, cfg):
    """Bucket/sort/pad edges per core; all cores share the quota structure."""
    src = np.asarray(edge_src).astype(np.int64)
    dst = np.asarray(edge_dst).astype(np.int64)
    C, CH, NT, NP, NDST = cfg.C, cfg.CH, cfg.NT, cfg.NP, cfg.NDST

    percore = []
    counts = []
    for c in range(C):
        m = (dst // NDST) == c
        s = src[m]
        dl = dst[m] - c * NDST
        p = s // CH
        o = np.lexsort((dl, p))
        s, dl, p = s[o], dl[o], p[o]
        t = dl >> 7
        cnt = np.bincount(p * NT + t, minlength=NP * NT).reshape(NP, NT)
        percore.append((s, dl, p, t))
        counts.append(cnt)

    quota = np.maximum.reduce(counts)
    quota = (quota + TILE - 1) // TILE * TILE  # pad runs to group multiples
    qflat = quota.reshape(-1)
    offs = np.concatenate([[0], np.cumsum(qflat)])
    T = int(offs[-1])
    offs_flat = offs[:-1].reshape(NP, NT)
    Lp = quota.sum(axis=1)

    # batches: per pass, chunks of BSZ stream positions (last one ragged)
    batches = []  # list of (pass, global_offset, nb)
    pass_base = np.concatenate([[0], np.cumsum(Lp)])
    for p in range(NP):
        off = 0
        while off < Lp[p]:
            nb = int(min(cfg.BSZ, Lp[p] - off))
            batches.append((p, int(pass_base[p] + off), nb))
            off += nb

    # group -> tile map + run boundary flags (shared across cores)
    NG = T // TILE
    group_tile = np.zeros(NG, np.int32)
    group_first = np.zeros(NG, bool)
    group_last = np.zeros(NG, bool)
    for p in range(NP):
        for t in range(NT):
            q = quota[p, t]
            if q == 0:
                continue
            g0 = offs_flat[p, t] // TILE
            g1 = g0 + q // TILE
            group_tile[g0:g1] = t
            group_first[g0] = True
            group_last[g1 - 1] = True

    per_core_arrays = []
    for c in range(C):
        s, dl, p, t = percore[c]
        key = p * NT + t
        first = np.searchsorted(key, np.arange(NP * NT), side="left")
        rank = np.arange(len(key)) - first[key]
        pos = offs_flat[p, t] + rank
        srcl = np.zeros(T, np.int16)
        drel = np.full(T, 200.0, np.float32)  # sentinel: never matches iota 0..127
        srcl[pos] = (s - p * CH).astype(np.int16)
        drel[pos] = (dl - (t << 7)).astype(np.float32)

        deg = np.bincount(dl, minlength=NDST).astype(np.float32)
        deg = np.maximum(deg, 1.0)
        degp = np.ones(NT * TILE, np.float32)
        degp[:NDST] = deg
        deg_arr = degp.reshape(NT, TILE).T.copy()  # [128, NT]

        idxw = np.tile(srcl.reshape(T // 16, 16).T, (8, 1)).copy()  # [128, T/16]
        drw = drel.reshape(T // TILE, TILE).T.astype(BF).copy()  # [128, T/128]
        per_core_arrays.append(dict(idxs=idxw, drel=drw, deg=deg_arr))

    structure = dict(
        T=T,
        NG=NG,
        batches=tuple(batches),
        group_tile=tuple(int(v) for v in group_tile),
        group_first=tuple(bool(v) for v in group_first),
        group_last=tuple(bool(v) for v in group_last),
    )
    return structure, per_core_arrays


def build_program(cfg, structure):
    N, D, C, CH, NT, NP = cfg.N, cfg.D, cfg.C, cfg.CH, cfg.NT, cfg.NP
    D2 = cfg.D2
    NDST = cfg.NDST
    T = structure["T"]
    batches = structure["batches"]
    group_tile = structure["group_tile"]
    group_first = structure["group_first"]
    group_last = structure["group_last"]
    OH_GROUPS = 16  # one-hot groups built per DVE op
    Relu = mybir.ActivationFunctionType.Relu
    Copy = mybir.ActivationFunctionType.Copy
    Sigmoid = mybir.ActivationFunctionType.Sigmoid

    nc = bacc.Bacc(None, target_bir_lowering=False, num_swdge_queues=4)
    x0 = nc.dram_tensor("x0", [N, FPAD], BF16, kind="ExternalInput")
    idxs_d = nc.dram_tensor("idxs", [128, T // 16], I16, kind="ExternalInput")
    drel_d = nc.dram_tensor("drel", [128, T // TILE], BF16, kind="ExternalInput")
    deg_d = nc.dram_tensor("deg", [128, NT], F32, kind="ExternalInput")
    w1_d = nc.dram_tensor("w1", [D, D], BF16, kind="ExternalInput")
    b1_d = nc.dram_tensor("b1", [1, D], BF16, kind="ExternalInput")
    w2_d = nc.dram_tensor("w2", [D, D2], BF16, kind="ExternalInput")
    b2_d = nc.dram_tensor("b2", [1, D2], BF16, kind="ExternalInput")
    wdbd_d = nc.dram_tensor("wdbd", [1, 2], F32, kind="ExternalInput")
    iota_d = nc.dram_tensor("iota", [128, OH_GROUPS * TILE], BF16, kind="ExternalInput")
    ident_d = nc.dram_tensor("ident", [128, 128], BF16, kind="ExternalInput")
    onesb_d = nc.dram_tensor("onesb", [1, 128], BF16, kind="ExternalInput")
    ones_d = nc.dram_tensor("ones1", [1, 128], F32, kind="ExternalInput")
    outp = nc.dram_tensor("out", [NDST, 1], F32, kind="ExternalOutput")
    x1loc = nc.dram_tensor("x1loc", [NDST, FPAD], BF16)
    x1full = nc.dram_tensor("x1full", [N, FPAD], BF16, addr_space="Shared")
    x1cps = [nc.dram_tensor(f"x1cp{p}", [CH, FPAD], BF16) for p in range(NP)]

    NFULL = NDST // TILE  # full dst tiles
    REM = NDST - NFULL * TILE  # lanes in the last (partial) tile, 0 if none

    with tile.TileContext(nc) as tc:
        with (
            tc.tile_pool(name="const", bufs=1) as cp,
            tc.tile_pool(name="work", bufs=10) as wp,
            tc.tile_pool(name="ohp", bufs=8) as ohp,
            tc.tile_pool(name="psacc", bufs=4, space="PSUM") as ps_acc,
            tc.tile_pool(name="pst", bufs=2, space="PSUM") as ps_t,
            tc.tile_pool(name="psm", bufs=2, space="PSUM") as ps_m,
        ):
            # ---- constants into SBUF ----
            iota_sb = cp.tile([128, OH_GROUPS * TILE], BF16)
            nc.sync.dma_start(iota_sb[:], iota_d[:, :])
            ident_sb = cp.tile([128, 128], BF16)
            nc.sync.dma_start(ident_sb[:], ident_d[:, :])
            onesb_sb = cp.tile([1, 128], BF16)
            nc.sync.dma_start(onesb_sb[:], onesb_d[:, :])
            ones_sb = cp.tile([1, 128], F32)
            nc.sync.dma_start(ones_sb[:], ones_d[:, :])
            w1_sb = cp.tile([D, D], BF16)
            nc.sync.dma_start(w1_sb[:], w1_d[:, :])
            b1_sb = cp.tile([1, D], BF16)
            nc.sync.dma_start(b1_sb[:], b1_d[:, :])
            w2_sb = cp.tile([D, D2], BF16)
            nc.sync.dma_start(w2_sb[:], w2_d[:, :])
            b2_sb = cp.tile([1, D2], BF16)
            nc.sync.dma_start(b2_sb[:], b2_d[:, :])
            wdbd_sb = cp.tile([1, 2], F32)
            nc.sync.dma_start(wdbd_sb[:], wdbd_d[:, :])
            deg_sb = cp.tile([128, NT], F32)
            nc.sync.dma_start(deg_sb[:], deg_d[:, :])

            rdeg = cp.tile([128, NT], F32)
            nc.vector.reciprocal(rdeg[:], deg_sb[:])

            idx_all = cp.tile([128, T // 16], I16)
            nc.sync.dma_start(idx_all[:], idxs_d[:, :])
            dr_all = cp.tile([128, T // TILE], BF16)
            nc.sync.dma_start(dr_all[:], drel_d[:, :])

            def pe_fence(*aps):
                for ap in aps:
                    with tc.tile_critical():
                        nop = nc.tensor.nop(hint="dep").ins
                        nop.ins = [nc.tensor.lower_ap(ap)]

            # broadcast Wd/32 and bd across partitions via a K=1 matmul
            pe_fence(ones_sb[:], wdbd_sb[:])
            wb_ps = ps_m.tile([128, 64], F32, tag="mm", name="wb_ps")
            nc.tensor.matmul(wb_ps[:, :2], lhsT=ones_sb[:], rhs=wdbd_sb[:],
                             start=True, stop=True)
            wb_rep = cp.tile([128, 2], F32)
            nc.scalar.activation(wb_rep[:], wb_ps[:, :2], Copy)
            nc.vector.tensor_scalar_mul(wb_rep[:, 0:1], wb_rep[:, 0:1], 1.0 / 32.0)

            agg = cp.tile([128, NT * D], F32)
            x1sb = cp.tile([128, NT * D], BF16)
            res = cp.tile([128, NT], F32)

            # last group (stream-wise) contributing to each tile
            last_gg_of_tile = {}
            for gg in range(len(group_tile)):
                if group_last[gg]:
                    last_gg_of_tile[group_tile[gg]] = gg

            def do_layer(tables, last, on_tile_done=None):
                nc.vector.memset(agg[:], 0.0)
                cur_ps = [None]

                def tail(t):
                    scaled = wp.tile([128, D], BF16, tag="scaled")
                    nc.scalar.activation(
                        scaled[:], agg[:, t * D:(t + 1) * D], Copy,
                        scale=rdeg[:, t:t + 1])
                    tps = ps_t.tile([D, 128], BF16, tag="tps")
                    nc.tensor.transpose(tps[:], scaled[:], ident_sb[:])
                    aggT = wp.tile([D, 128], BF16, tag="aggT")
                    nc.scalar.activation(aggT[:], tps[:], Copy)
                    if not last:
                        x1ps = ps_m.tile([128, D], F32, tag="mm", name="x1ps")
                        nc.tensor.matmul(x1ps[:], lhsT=aggT[:], rhs=w1_sb[:],
                                         start=True, stop=False)
                        nc.tensor.matmul(x1ps[:], lhsT=onesb_sb[:], rhs=b1_sb[:],
                                         start=False, stop=True)
                        nc.scalar.activation(
                            x1sb[:, t * D:(t + 1) * D], x1ps[:], Relu)
                    else:
                        x2ps = ps_m.tile([128, D], F32, tag="mm", name="x2ps")
                        nc.tensor.matmul(x2ps[:, :D2], lhsT=aggT[:], rhs=w2_sb[:],
                                         start=True, stop=False)
                        nc.tensor.matmul(x2ps[:, :D2], lhsT=onesb_sb[:], rhs=b2_sb[:],
                                         start=False, stop=True)
                        x2sb = wp.tile([128, D2], BF16, tag="x2sb")
                        ssb = wp.tile([128, 1], F32, tag="ssb")
                        nc.scalar.activation(x2sb[:], x2ps[:, :D2], Relu,
                                             accum_out=ssb[:])
                        nc.scalar.activation(
                            res[:, t:t + 1], ssb[:], Sigmoid,
                            bias=wb_rep[:, 1:2], scale=wb_rep[:, 0:1])

                done = [False] * NT
                prefix = [0]  # tiles 0..prefix-1 all done

                def mark_done(t):
                    tail(t)
                    done[t] = True
                    while prefix[0] < NT and done[prefix[0]]:
                        prefix[0] += 1
                    if on_tile_done is not None:
                        on_tile_done(prefix[0])

                for bi, (p, boff, nb) in enumerate(batches):
                    ncol = nb // TILE
                    idx_t = idx_all[:, boff // 16:(boff + nb) // 16]
                    dr_t = dr_all[:, boff // TILE:(boff + nb) // TILE]
                    msgs = wp.tile([128, ncol * FPAD], BF16, tag="msgs")
                    msgs3 = msgs[:].rearrange("p (c f) -> p c f", f=FPAD)
                    nc.gpsimd.dma_gather(
                        msgs3,
                        tables[p],
                        idx_t,
                        nb,
                        nb,
                        FPAD,
                        queue_num=bi % 4,
                    )
                    nsub = _cdiv(ncol, OH_GROUPS)
                    for sc in range(nsub):
                        gcols = min(OH_GROUPS, ncol - sc * OH_GROUPS)
                        m = gcols * TILE
                        oh = ohp.tile([128, OH_GROUPS * TILE], BF16, tag="oh")
                        in1 = (
                            dr_t[:, sc * OH_GROUPS: sc * OH_GROUPS + gcols]
                            .rearrange("p (g o) -> p g o", o=1)
                            .to_broadcast([128, gcols, TILE])
                        )
                        nc.vector.tensor_tensor(
                            out=oh[:, :m],
                            in0=iota_sb[:, :m],
                            in1=in1,
                            op=mybir.AluOpType.is_equal,
                        )
                        for g in range(gcols):
                            gg = boff // TILE + sc * OH_GROUPS + g
                            t = group_tile[gg]
                            if group_first[gg]:
                                cur_ps[0] = ps_acc.tile(
                                    [128, D], F32, tag="acc", name="accps")
                            nc.tensor.matmul(
                                cur_ps[0][:],
                                lhsT=oh[:, g * TILE:(g + 1) * TILE],
                                rhs=msgs[:, (sc * OH_GROUPS + g) * FPAD:
                                         (sc * OH_GROUPS + g) * FPAD + D],
                                start=group_first[gg],
                                stop=group_last[gg],
                            )
                            if group_last[gg]:
                                nc.vector.tensor_add(
                                    agg[:, t * D:(t + 1) * D],
                                    agg[:, t * D:(t + 1) * D],
                                    cur_ps[0][:],
                                )
                                if last_gg_of_tile[t] == gg:
                                    mark_done(t)

                for t in range(NT):
                    if not done[t]:
                        mark_done(t)

            # ---------------- layer 1 ----------------
            # x1sb -> x1loc written in chunks as tiles complete so the
            # AllGather can start right after the last tail
            WB = 24
            wb_done = [0]  # next tile index to write back

            def on_tile_done(pref):
                while wb_done[0] + WB <= min(pref, NFULL):
                    a = wb_done[0]
                    b = min(a + WB, NFULL)
                    nc.sync.dma_start(
                        x1loc[a * TILE: b * TILE, :D].rearrange(
                            "(t r) f -> r t f", r=TILE),
                        x1sb[:, a * D: b * D].rearrange("p (t f) -> p t f", f=D),
                    )
                    wb_done[0] = b

            do_layer([x0[p * CH:(p + 1) * CH, :] for p in range(NP)],
                     last=False, on_tile_done=on_tile_done)

            if wb_done[0] < NFULL:
                a = wb_done[0]
                nc.sync.dma_start(
                    x1loc[a * TILE: NFULL * TILE, :D].rearrange(
                        "(t r) f -> r t f", r=TILE),
                    x1sb[:, a * D: NFULL * D].rearrange("p (t f) -> p t f", f=D),
                )
            if REM:
                nc.sync.dma_start(
                    x1loc[NFULL * TILE:, :D],
                    x1sb[:REM, NFULL * D:(NFULL + 1) * D],
                )
            if cfg.no_cc:
                nc.sync.dma_start(x1full[:NDST, :], x1loc[:, :])
            else:
                nc.gpsimd.collective_compute(
                    "AllGather",
                    mybir.AluOpType.bypass,
                    replica_groups=[list(range(C))],
                    ins=[x1loc[:, :]],
                    outs=[x1full[:, :]],
                )

            # ---------------- layer 2 + head ----------------
            # copy out of the Shared address space (chunked so pass 0 can
            # start gathering after only its chunk has landed)
            for p in range(NP):
                nc.sync.dma_start(
                    x1cps[p][:, :], x1full[p * CH:(p + 1) * CH, :])
            do_layer([t[:, :] for t in x1cps], last=True)

            if NFULL:
                nc.sync.dma_start(
                    outp[: NFULL * TILE, :].rearrange("(t r) o -> r (t o)", r=TILE),
                    res[:, :NFULL],
                )
            if REM:
                nc.sync.dma_start(
                    outp[NFULL * TILE:, :],
                    res[:REM, NFULL:NFULL + 1],
                )

    nc.finalize()
    return nc


_CACHE = {}


def _get_program(cfg, structure):
    key = (cfg.N, cfg.D, cfg.C, cfg.CH, cfg.BSZ, cfg.no_cc,
           structure["T"], structure["batches"], structure["group_tile"],
           structure["group_first"], structure["group_last"])
    if key not in _CACHE:
        _CACHE[key] = build_program(cfg, structure)
    return _CACHE[key]


OH_GROUPS = 16

# exposed for test.py to rerun with tracing without rebuilding
LAST_RUN = {}


def kernel(node_features, edge_src, edge_dst, W1, b1, W2, b2, Wd, bd,
           cfg=None, trace=False):
    cfg = cfg or Cfg(N=node_features.shape[0])
    structure, per_core = plan_edges(edge_src, edge_dst, cfg)
    nc = _get_program(cfg, structure)

    N = cfg.N
    x0 = np.zeros((N, FPAD), dtype=BF)
    x0[:, :cfg.D] = np.asarray(node_features, dtype=np.float32).astype(BF)
    iota = np.tile(np.arange(128, dtype=np.float32), OH_GROUPS)[None, :].repeat(
        128, axis=0).astype(BF)
    ident = np.eye(128, dtype=np.float32).astype(BF)
    onesb = np.ones((1, 128), BF)
    ones1 = np.ones((1, 128), np.float32)
    wdbd = np.array([[np.asarray(Wd).reshape(-1)[0],
                      np.asarray(bd).reshape(-1)[0]]], np.float32)
    shared = dict(
        x0=x0,
        w1=np.asarray(W1, np.float32).astype(BF),
        b1=np.asarray(b1, np.float32).reshape(1, -1).astype(BF),
        w2=np.asarray(W2, np.float32).astype(BF),
        b2=np.asarray(b2, np.float32).reshape(1, -1).astype(BF),
        wdbd=wdbd,
        iota=iota,
        ident=ident,
        onesb=onesb,
        ones1=ones1,
    )
    in_maps = []
    for c in range(cfg.C):
        m = dict(shared)
        m.update(per_core[c])
        in_maps.append(m)

    core_ids = list(range(cfg.C))
    r = run_bass_kernel_spmd(nc, in_maps, core_ids, trace=trace)
    LAST_RUN["nc"] = nc
    LAST_RUN["in_maps"] = in_maps
    LAST_RUN["results"] = r
    out = np.concatenate([r.results[c]["out"] for c in range(cfg.C)], axis=0)
    return out


# revision 19
# speedup vs baseline: 1.2342x; 1.2342x over previous
"""Two-layer GraphConv (gather + segment-mean + linear + ReLU) x2 + sigmoid head,
distributed over 8 NeuronCores.

Sharding: destination nodes are partitioned across the 8 cores (12.5k each).
Host-side prep (pure index work): each core's edges are bucketed by
(src-chunk-of-25k, dst), each (chunk x dst-tile-of-128) run is padded to a
multiple of 128 with sentinel edges so all 8 cores share one SPMD program.

On device, per layer (bf16 data path, fp32 accumulation in PSUM):
  - node features live in DRAM as [N, 128] bf16 (64 real features + 64 pad)
    so one 256B dma_gather element fetches one row with no conversion step
  - dma_gather fetches rows via int16 chunk-local indices (4 swdge queues,
    1024-desc batches, 10 in flight; the whole idx/drel stream is preloaded
    into SBUF once and shared by both layers)
  - one-hot matrices are built on the vector engine in bf16
  - TensorE bf16 matmuls (lhsT=one-hot, rhs=gathered msgs[:, :64]) segment-sum
    into PSUM, accumulated per dst-tile into an fp32 SBUF accumulator
  - per-tile tails (1/deg scale on ACT, PE-transpose, fused W+bias bf16
    matmuls, ReLU) are emitted inline as soon as a tile's last run finishes,
    so they overlap the remaining gather/matmul stream; x1 writeback is
    chunked the same way so the AllGather starts early
  - AllGather of x1 (bf16, padded layout); layer 2 gathers from per-pass
    local-DRAM copies (reads from the Shared address space are ~70% slower)
  - layer-2 tail: ReLU with accumulated row-sum, sigmoid(scale*s+bias)

Measured on 8 trn2 cores: 1.97-2.05 ms HW exec (baseline fp32 dma-gather
implementation: 12.2 ms); rel err vs fp32 reference ~1.7e-5. The pacer is
the swdge gather itself (~2.8 us per 1024-desc batch = HBM random-read
bound); PE pipelines groups at ~44 ns within a sub-batch and is ~20% busy.
BSZ=2048 overflows the per-queue descriptor ring (~2.5k-4k desc cap) and
crashes the device - do not raise BSZ or msgs bufs beyond 10.
"""

import os
import sys

for _p in ("/opt/trn_rl_repo", "/opt/pypackages"):
    if _p not in sys.path and os.path.isdir(_p):
        sys.path.insert(0, _p)

import numpy as np
import ml_dtypes

from concourse import bacc, bass, mybir, tile
from concourse.bass_utils import run_bass_kernel_spmd

F32 = mybir.dt.float32
BF16 = mybir.dt.bfloat16
I16 = mybir.dt.int16
BF = ml_dtypes.bfloat16

TILE = 128
FPAD = 128  # padded feature row width (bf16) -> 256B gather element


def _cdiv(a, b):
    return (a + b - 1) // b


class Cfg:
    def __init__(self, N=100000, D=64, C=8, CH=25000, BSZ=1024, no_cc=False):
        self.no_cc = no_cc
        assert N % C == 0 and N % CH == 0
        assert CH <= 32768  # int16 gather indices
        assert BSZ % 128 == 0
        self.N, self.D, self.C, self.CH, self.BSZ = N, D, C, CH, BSZ
        self.NDST = N // C
        self.NT = _cdiv(self.NDST, TILE)
        self.NP = N // CH
        self.D2 = 32  # layer-2 output width


def plan_edges(edge_src, edge_dst, cfg):
    """Bucket/sort/pad edges per core; all cores share the quota structure."""
    src = np.asarray(edge_src).astype(np.int64)
    dst = np.asarray(edge_dst).astype(np.int64)
    C, CH, NT, NP, NDST = cfg.C, cfg.CH, cfg.NT, cfg.NP, cfg.NDST

    percore = []
    counts = []
    for c in range(C):
        m = (dst // NDST) == c
        s = src[m]
        dl = dst[m] - c * NDST
        p = s // CH
        o = np.lexsort((dl, p))
        s, dl, p = s[o], dl[o], p[o]
        t = dl >> 7
        cnt = np.bincount(p * NT + t, minlength=NP * NT).reshape(NP, NT)
        percore.append((s, dl, p, t))
        counts.append(cnt)

    quota = np.maximum.reduce(counts)
    quota = (quota + TILE - 1) // TILE * TILE  # pad runs to group multiples
    qflat = quota.reshape(-1)
    offs = np.concatenate([[0], np.cumsum(qflat)])
    T = int(offs[-1])
    offs_flat = offs[:-1].reshape(NP, NT)
    Lp = quota.sum(axis=1)

    # batches: per pass, chunks of BSZ stream positions (last one ragged)
    batches = []  # list of (pass, global_offset, nb)
    pass_base = np.concatenate([[0], np.cumsum(Lp)])
    for p in range(NP):
        off = 0
        while off < Lp[p]:
            nb = int(min(cfg.BSZ, Lp[p] - off))
            batches.append((p, int(pass_base[p] + off), nb))
            off += nb

    # group -> tile map + run boundary flags (shared across cores)
    NG = T // TILE
    group_tile = np.zeros(NG, np.int32)
    group_first = np.zeros(NG, bool)
    group_last = np.zeros(NG, bool)
    for p in range(NP):
        for t in range(NT):
            q = quota[p, t]
            if q == 0:
                continue
            g0 = offs_flat[p, t] // TILE
            g1 = g0 + q // TILE
            group_tile[g0:g1] = t
            group_first[g0] = True
            group_last[g1 - 1] = True

    per_core_arrays = []
    for c in range(C):
        s, dl, p, t = percore[c]
        key = p * NT + t
        first = np.searchsorted(key, np.arange(NP * NT), side="left")
        rank = np.arange(len(key)) - first[key]
        pos = offs_flat[p, t] + rank
        srcl = np.zeros(T, np.int16)
        drel = np.full(T, 200.0, np.float32)  # sentinel: never matches iota 0..127
        srcl[pos] = (s - p * CH).astype(np.int16)
        drel[pos] = (dl - (t << 7)).astype(np.float32)

        deg = np.bincount(dl, minlength=NDST).astype(np.float32)
        deg = np.maximum(deg, 1.0)
        degp = np.ones(NT * TILE, np.float32)
        degp[:NDST] = deg
        deg_arr = degp.reshape(NT, TILE).T.copy()  # [128, NT]

        idxw = np.tile(srcl.reshape(T // 16, 16).T, (8, 1)).copy()  # [128, T/16]
        drw = drel.reshape(T // TILE, TILE).T.astype(BF).copy()  # [128, T/128]
        per_core_arrays.append(dict(idxs=idxw, drel=drw, deg=deg_arr))

    structure = dict(
        T=T,
        NG=NG,
        batches=tuple(batches),
        group_tile=tuple(int(v) for v in group_tile),
        group_first=tuple(bool(v) for v in group_first),
        group_last=tuple(bool(v) for v in group_last),
    )
    return structure, per_core_arrays


def build_program(cfg, structure):
    N, D, C, CH, NT, NP = cfg.N, cfg.D, cfg.C, cfg.CH, cfg.NT, cfg.NP
    D2 = cfg.D2
    NDST = cfg.NDST
    T = structure["T"]
    batches = structure["batches"]
    group_tile = structure["group_tile"]
    group_first = structure["group_first"]
    group_last = structure["group_last"]
    OH_GROUPS = 16  # one-hot groups built per DVE op
    Relu = mybir.ActivationFunctionType.Relu
    Copy = mybir.ActivationFunctionType.Copy
    Sigmoid = mybir.ActivationFunctionType.Sigmoid

    nc = bacc.Bacc(None, target_bir_lowering=False, num_swdge_queues=4)
    x0 = nc.dram_tensor("x0", [N, FPAD], BF16, kind="ExternalInput")
    idxs_d = nc.dram_tensor("idxs", [128, T // 16], I16, kind="ExternalInput")
    drel_d = nc.dram_tensor("drel", [128, T // TILE], BF16, kind="ExternalInput")
    deg_d = nc.dram_tensor("deg", [128, NT], F32, kind="ExternalInput")
    w1_d = nc.dram_tensor("w1", [D, D], BF16, kind="ExternalInput")
    b1_d = nc.dram_tensor("b1", [1, D], BF16, kind="ExternalInput")
    w2_d = nc.dram_tensor("w2", [D, D2], BF16, kind="ExternalInput")
    b2_d = nc.dram_tensor("b2", [1, D2], BF16, kind="ExternalInput")
    wdbd_d = nc.dram_tensor("wdbd", [1, 2], F32, kind="ExternalInput")
    iota_d = nc.dram_tensor("iota", [128, OH_GROUPS * TILE], BF16, kind="ExternalInput")
    ident_d = nc.dram_tensor("ident", [128, 128], BF16, kind="ExternalInput")
    onesb_d = nc.dram_tensor("onesb", [1, 128], BF16, kind="ExternalInput")
    ones_d = nc.dram_tensor("ones1", [1, 128], F32, kind="ExternalInput")
    outp = nc.dram_tensor("out", [NDST, 1], F32, kind="ExternalOutput")
    x1loc = nc.dram_tensor("x1loc", [NDST, FPAD], BF16)
    x1full = nc.dram_tensor("x1full", [N, FPAD], BF16, addr_space="Shared")
    x1cps = [nc.dram_tensor(f"x1cp{p}", [CH, FPAD], BF16) for p in range(NP)]
    x0cps = [nc.dram_tensor(f"x0cp{p}", [CH, FPAD], BF16) for p in range(NP)]

    NFULL = NDST // TILE  # full dst tiles
    REM = NDST - NFULL * TILE  # lanes in the last (partial) tile, 0 if none

    with tile.TileContext(nc) as tc:
        with (
            tc.tile_pool(name="const", bufs=1) as cp,
            tc.tile_pool(name="work", bufs=10) as wp,
            tc.tile_pool(name="ohp", bufs=8) as ohp,
            tc.tile_pool(name="psacc", bufs=4, space="PSUM") as ps_acc,
            tc.tile_pool(name="pst", bufs=2, space="PSUM") as ps_t,
            tc.tile_pool(name="psm", bufs=2, space="PSUM") as ps_m,
        ):
            # ---- constants into SBUF ----
            iota_sb = cp.tile([128, OH_GROUPS * TILE], BF16)
            nc.sync.dma_start(iota_sb[:], iota_d[:, :])
            ident_sb = cp.tile([128, 128], BF16)
            nc.sync.dma_start(ident_sb[:], ident_d[:, :])
            onesb_sb = cp.tile([1, 128], BF16)
            nc.sync.dma_start(onesb_sb[:], onesb_d[:, :])
            ones_sb = cp.tile([1, 128], F32)
            nc.sync.dma_start(ones_sb[:], ones_d[:, :])
            w1_sb = cp.tile([D, D], BF16)
            nc.sync.dma_start(w1_sb[:], w1_d[:, :])
            b1_sb = cp.tile([1, D], BF16)
            nc.sync.dma_start(b1_sb[:], b1_d[:, :])
            w2_sb = cp.tile([D, D2], BF16)
            nc.sync.dma_start(w2_sb[:], w2_d[:, :])
            b2_sb = cp.tile([1, D2], BF16)
            nc.sync.dma_start(b2_sb[:], b2_d[:, :])
            wdbd_sb = cp.tile([1, 2], F32)
            nc.sync.dma_start(wdbd_sb[:], wdbd_d[:, :])
            deg_sb = cp.tile([128, NT], F32)
            nc.sync.dma_start(deg_sb[:], deg_d[:, :])

            rdeg = cp.tile([128, NT], F32)
            nc.vector.reciprocal(rdeg[:], deg_sb[:])

            idx_all = cp.tile([128, T // 16], I16)
            nc.sync.dma_start(idx_all[:], idxs_d[:, :])
            dr_all = cp.tile([128, T // TILE], BF16)
            nc.sync.dma_start(dr_all[:], drel_d[:, :])

            def pe_fence(*aps):
                for ap in aps:
                    with tc.tile_critical():
                        nop = nc.tensor.nop(hint="dep").ins
                        nop.ins = [nc.tensor.lower_ap(ap)]

            # broadcast Wd/32 and bd across partitions via a K=1 matmul
            pe_fence(ones_sb[:], wdbd_sb[:])
            wb_ps = ps_m.tile([128, 64], F32, tag="mm", name="wb_ps")
            nc.tensor.matmul(wb_ps[:, :2], lhsT=ones_sb[:], rhs=wdbd_sb[:],
                             start=True, stop=True)
            wb_rep = cp.tile([128, 2], F32)
            nc.scalar.activation(wb_rep[:], wb_ps[:, :2], Copy)
            nc.vector.tensor_scalar_mul(wb_rep[:, 0:1], wb_rep[:, 0:1], 1.0 / 32.0)

            agg = cp.tile([128, NT * D], F32)
            x1sb = cp.tile([128, NT * D], BF16)
            res = cp.tile([128, NT], F32)

            # last group (stream-wise) contributing to each tile
            last_gg_of_tile = {}
            for gg in range(len(group_tile)):
                if group_last[gg]:
                    last_gg_of_tile[group_tile[gg]] = gg

            def do_layer(tables, last, on_tile_done=None):
                nc.vector.memset(agg[:], 0.0)
                cur_ps = [None]

                def tail(t):
                    scaled = wp.tile([128, D], BF16, tag="scaled")
                    nc.scalar.activation(
                        scaled[:], agg[:, t * D:(t + 1) * D], Copy,
                        scale=rdeg[:, t:t + 1])
                    tps = ps_t.tile([D, 128], BF16, tag="tps")
                    nc.tensor.transpose(tps[:], scaled[:], ident_sb[:])
                    aggT = wp.tile([D, 128], BF16, tag="aggT")
                    nc.scalar.activation(aggT[:], tps[:], Copy)
                    if not last:
                        x1ps = ps_m.tile([128, D], F32, tag="mm", name="x1ps")
                        nc.tensor.matmul(x1ps[:], lhsT=aggT[:], rhs=w1_sb[:],
                                         start=True, stop=False)
                        nc.tensor.matmul(x1ps[:], lhsT=onesb_sb[:], rhs=b1_sb[:],
                                         start=False, stop=True)
                        nc.scalar.activation(
                            x1sb[:, t * D:(t + 1) * D], x1ps[:], Relu)
                    else:
                        x2ps = ps_m.tile([128, D], F32, tag="mm", name="x2ps")
                        nc.tensor.matmul(x2ps[:, :D2], lhsT=aggT[:], rhs=w2_sb[:],
                                         start=True, stop=False)
                        nc.tensor.matmul(x2ps[:, :D2], lhsT=onesb_sb[:], rhs=b2_sb[:],
                                         start=False, stop=True)
                        x2sb = wp.tile([128, D2], BF16, tag="x2sb")
                        ssb = wp.tile([128, 1], F32, tag="ssb")
                        nc.scalar.activation(x2sb[:], x2ps[:, :D2], Relu,
                                             accum_out=ssb[:])
                        nc.scalar.activation(
                            res[:, t:t + 1], ssb[:], Sigmoid,
                            bias=wb_rep[:, 1:2], scale=wb_rep[:, 0:1])

                done = [False] * NT
                prefix = [0]  # tiles 0..prefix-1 all done

                def mark_done(t):
                    tail(t)
                    done[t] = True
                    while prefix[0] < NT and done[prefix[0]]:
                        prefix[0] += 1
                    if on_tile_done is not None:
                        on_tile_done(prefix[0])

                for bi, (p, boff, nb) in enumerate(batches):
                    ncol = nb // TILE
                    idx_t = idx_all[:, boff // 16:(boff + nb) // 16]
                    dr_t = dr_all[:, boff // TILE:(boff + nb) // TILE]
                    msgs = wp.tile([128, ncol * FPAD], BF16, tag="msgs")
                    msgs3 = msgs[:].rearrange("p (c f) -> p c f", f=FPAD)
                    nc.gpsimd.dma_gather(
                        msgs3,
                        tables[p],
                        idx_t,
                        nb,
                        nb,
                        FPAD,
                        queue_num=bi % 4,
                    )
                    nsub = _cdiv(ncol, OH_GROUPS)
                    for sc in range(nsub):
                        gcols = min(OH_GROUPS, ncol - sc * OH_GROUPS)
                        m = gcols * TILE
                        oh = ohp.tile([128, OH_GROUPS * TILE], BF16, tag="oh")
                        in1 = (
                            dr_t[:, sc * OH_GROUPS: sc * OH_GROUPS + gcols]
                            .rearrange("p (g o) -> p g o", o=1)
                            .to_broadcast([128, gcols, TILE])
                        )
                        nc.vector.tensor_tensor(
                            out=oh[:, :m],
                            in0=iota_sb[:, :m],
                            in1=in1,
                            op=mybir.AluOpType.is_equal,
                        )
                        for g in range(gcols):
                            gg = boff // TILE + sc * OH_GROUPS + g
                            t = group_tile[gg]
                            if group_first[gg]:
                                cur_ps[0] = ps_acc.tile(
                                    [128, D], F32, tag="acc", name="accps")
                            nc.tensor.matmul(
                                cur_ps[0][:],
                                lhsT=oh[:, g * TILE:(g + 1) * TILE],
                                rhs=msgs[:, (sc * OH_GROUPS + g) * FPAD:
                                         (sc * OH_GROUPS + g) * FPAD + D],
                                start=group_first[gg],
                                stop=group_last[gg],
                            )
                            if group_last[gg]:
                                nc.vector.tensor_add(
                                    agg[:, t * D:(t + 1) * D],
                                    agg[:, t * D:(t + 1) * D],
                                    cur_ps[0][:],
                                )
                                if last_gg_of_tile[t] == gg:
                                    mark_done(t)

                for t in range(NT):
                    if not done[t]:
                        mark_done(t)

            # copy the input table out of its ExternalInput placement into
            # local DRAM chunks (input-region reads can be much slower);
            # pass p's gathers gate only on chunk p's copy
            for p in range(NP):
                nc.sync.dma_start(x0cps[p][:, :], x0[p * CH:(p + 1) * CH, :])

            # ---------------- layer 1 ----------------
            # x1sb -> x1loc written in chunks as tiles complete so the
            # AllGather can start right after the last tail
            WB = 24
            wb_done = [0]  # next tile index to write back

            def on_tile_done(pref):
                while wb_done[0] + WB <= min(pref, NFULL):
                    a = wb_done[0]
                    b = min(a + WB, NFULL)
                    nc.sync.dma_start(
                        x1loc[a * TILE: b * TILE, :D].rearrange(
                            "(t r) f -> r t f", r=TILE),
                        x1sb[:, a * D: b * D].rearrange("p (t f) -> p t f", f=D),
                    )
                    wb_done[0] = b

            do_layer([t[:, :] for t in x0cps],
                     last=False, on_tile_done=on_tile_done)

            if wb_done[0] < NFULL:
                a = wb_done[0]
                nc.sync.dma_start(
                    x1loc[a * TILE: NFULL * TILE, :D].rearrange(
                        "(t r) f -> r t f", r=TILE),
                    x1sb[:, a * D: NFULL * D].rearrange("p (t f) -> p t f", f=D),
                )
            if REM:
                nc.sync.dma_start(
                    x1loc[NFULL * TILE:, :D],
                    x1sb[:REM, NFULL * D:(NFULL + 1) * D],
                )
            if cfg.no_cc:
                nc.sync.dma_start(x1full[:NDST, :], x1loc[:, :])
            else:
                nc.gpsimd.collective_compute(
                    "AllGather",
                    mybir.AluOpType.bypass,
                    replica_groups=[list(range(C))],
                    ins=[x1loc[:, :]],
                    outs=[x1full[:, :]],
                )

            # ---------------- layer 2 + head ----------------
            # copy out of the Shared address space (chunked so pass 0 can
            # start gathering after only its chunk has landed)
            for p in range(NP):
                nc.sync.dma_start(
                    x1cps[p][:, :], x1full[p * CH:(p + 1) * CH, :])
            do_layer([t[:, :] for t in x1cps], last=True)

            if NFULL:
                nc.sync.dma_start(
                    outp[: NFULL * TILE, :].rearrange("(t r) o -> r (t o)", r=TILE),
                    res[:, :NFULL],
                )
            if REM:
                nc.sync.dma_start(
                    outp[NFULL * TILE:, :],
                    res[:REM, NFULL:NFULL + 1],
                )

    nc.finalize()
    return nc


_CACHE = {}


def _get_program(cfg, structure):
    key = (cfg.N, cfg.D, cfg.C, cfg.CH, cfg.BSZ, cfg.no_cc,
           structure["T"], structure["batches"], structure["group_tile"],
           structure["group_first"], structure["group_last"])
    if key not in _CACHE:
        _CACHE[key] = build_program(cfg, structure)
    return _CACHE[key]


OH_GROUPS = 16

# exposed for test.py to rerun with tracing without rebuilding
LAST_RUN = {}


def kernel(node_features, edge_src, edge_dst, W1, b1, W2, b2, Wd, bd,
           cfg=None, trace=False):
    cfg = cfg or Cfg(N=node_features.shape[0])
    structure, per_core = plan_edges(edge_src, edge_dst, cfg)
    nc = _get_program(cfg, structure)

    N = cfg.N
    x0 = np.zeros((N, FPAD), dtype=BF)
    x0[:, :cfg.D] = np.asarray(node_features, dtype=np.float32).astype(BF)
    iota = np.tile(np.arange(128, dtype=np.float32), OH_GROUPS)[None, :].repeat(
        128, axis=0).astype(BF)
    ident = np.eye(128, dtype=np.float32).astype(BF)
    onesb = np.ones((1, 128), BF)
    ones1 = np.ones((1, 128), np.float32)
    wdbd = np.array([[np.asarray(Wd).reshape(-1)[0],
                      np.asarray(bd).reshape(-1)[0]]], np.float32)
    shared = dict(
        x0=x0,
        w1=np.asarray(W1, np.float32).astype(BF),
        b1=np.asarray(b1, np.float32).reshape(1, -1).astype(BF),
        w2=np.asarray(W2, np.float32).astype(BF),
        b2=np.asarray(b2, np.float32).reshape(1, -1).astype(BF),
        wdbd=wdbd,
        iota=iota,
        ident=ident,
        onesb=onesb,
        ones1=ones1,
    )
    in_maps = []
    for c in range(cfg.C):
        m = dict(shared)
        m.update(per_core[c])
        in_maps.append(m)

    core_ids = list(range(cfg.C))
    r = run_bass_kernel_spmd(nc, in_maps, core_ids, trace=trace)
    LAST_RUN["nc"] = nc
    LAST_RUN["in_maps"] = in_maps
    LAST_RUN["results"] = r
    out = np.concatenate([r.results[c]["out"] for c in range(cfg.C)], axis=0)
    return out
